# revision 13
# baseline (speedup 1.0000x reference)
"""CycleFC (per-channel width-shift + 1x1 conv) Trainium2 kernel.

Full shapes: x [32, 256, 56, 56] f32, weight [256, 256], bias [256].
out[b,o,h,w] = sum_c weight[o,c] * shift(x)[b,c,h,w] + bias[o]
where shift moves channel c along width by off(c) = (c+3)%7-3, zero-padded.

Strategy
--------
- Data-parallel over batch: 8 cores x 4 batches.
- Channels are permuted host-side so equal-shift channels ("classes",
  c mod 7) are contiguous; the weight matrix is permuted to match, so the
  contraction is order-invariant and no output un-permutation is needed.
- Host-side layout prep: x is stored padded as [C, 56 rows x 59] with 3
  zero columns after each row's 56 data columns (flat per-channel blocks
  of 3304 floats, original channel order).  In this layout the
  per-channel width-shift is a single CONTIGUOUS flat shift:
    tile[p, h*59+ww] = x_pad[c, h*59 + ww + off]
  * ww+off in [56,59) reads the row's pad zeros  == correct zero padding
  * crossing a row boundary reads the previous/next row's pad zeros
  * off<0 head reads the previous channel block's last 3 elements, which
    are that channel's row-55 pad zeros (off<0 classes never start the
    buffer; off=0 channels come first in class order)
  * off>0 tail reads spill into the next channel's data, landing only in
    the tile's pad columns, which the matmul never reads.
  So each shift class is ONE contiguous DMA (~12.9 KB per partition) and
  there are no on-device memsets or edge fixups at all.  This replaces
  v1's per-row strided loads (~6k descriptor runs of ~212 B per tile)
  that made it SWDGE-descriptor-emission-bound (~700 us of Q7 time).
- Matmul: out[o, nsl] = lhsT.T @ rhs, lhsT = permuted weight.T [C, O],
  rhs = x tile rows [C, 8 rows x 56-of-59 cols] (N=448, small stride over
  the pad cols), accumulated over the 2 channel groups in PSUM.  Bias is
  fused into the PSUM->SBUF eviction on the scalar engine.
- fp32r (tf32-like, 4x PE throughput) by declaring the x/weight DRAM
  params float32r (same bits as f32) so plain DMAs produce f32r tiles.

Semaphore-wait budget (walrus encodes ONE wait per instruction)
---------------------------------------------------------------
- Class-load DMAs target fresh tiles, so they carry no data waits (a
  reused SWDGE lane spends its wait on an own-lane wait).
- Per tile, a chain of 1-element DVE probe reads (one per class DMA)
  folds the load lanes into the DVE clock; the first matmul of a tile
  then needs a single DVE wait covering all of them.
- Tiny absorber matmuls let PE observe the two weight-tile DMA lanes;
  a tiny activation lets ACT observe the bias lane, so evictions wait on
  PE alone and output stores (8 fresh HWDGE lanes) wait on ACT alone.
"""

import numpy as np

B, C, O, H, W = 32, 256, 256, 56, 56
KS = 7
PAD = KS // 2
N_CORES = 8
B_LOC = B // N_CORES
WP = W + PAD  # 59: padded row width
EPC = H * WP  # 3304: elements per channel block
ROWS_PER_CHUNK = 8
N_CHUNKS = H // ROWS_PER_CHUNK  # 7
USE_F32R = True

_OFFS = [(r + PAD) % KS - PAD for r in range(KS)]  # [0,1,2,3,-3,-2,-1]


def _perm_and_segments(n_chan):
    """Channel permutation (sort by c mod 7) and per-128-group DMA segments.

    Returns (perm, segs) where segs[g] is a list of
    (off, p0, p1, ch_start, ch_stop) : local partitions [p0,p1) of group g
    hold original channels ch_start:ch_stop:KS, shifted by `off`.
    """
    mods = np.arange(n_chan) % KS
    perm = np.argsort(mods, kind="stable")
    counts = [int(np.sum(mods == r)) for r in range(KS)]
    starts = np.concatenate([[0], np.cumsum(counts)])
    n_groups = n_chan // 128
    segs = [[] for _ in range(n_groups)]
    for r in range(KS):
        cs, ce = int(starts[r]), int(starts[r + 1])
        for g in range(n_groups):
            s, e = max(cs, g * 128), min(ce, (g + 1) * 128)
            if s >= e:
                continue
            j0, j1 = s - cs, e - cs
            segs[g].append((_OFFS[r], s - g * 128, e - g * 128,
                            r + KS * j0, r + KS * (j1 - 1) + 1))
    return perm, segs


def build_nc(b_loc=B_LOC, n_chan=C, n_out=O, h=H, w=W, rows_per_chunk=ROWS_PER_CHUNK,
             use_f32r=USE_F32R, psum_bufs=6):
    import concourse.bass as bass
    import concourse.mybir as mybir
    from concourse.tile import TileContext, add_dep_helper

    f32 = mybir.dt.float32
    xdt = mybir.dt.float32r if use_f32r else f32
    wp = w + PAD
    epc = h * wp
    hw = h * w
    n_free = rows_per_chunk * w
    n_chunks = h // rows_per_chunk
    assert h % rows_per_chunk == 0
    n_groups = n_chan // 128
    o_groups = n_out // 128
    _, segs = _perm_and_segments(n_chan)

    nc = bass.Bass()
    # x is the host-padded flat layout: per channel 56 rows x (56 data +
    # 3 zero) floats; +4 tail floats so off>0 spill reads stay in bounds.
    x_d = nc.declare_dram_parameter("x", [b_loc, n_chan * epc + 4], xdt,
                                    isOutput=False)
    w_d = nc.declare_dram_parameter("wt", [n_chan, n_out], xdt, isOutput=False)
    b_d = nc.declare_dram_parameter("bias", [128, o_groups], f32, isOutput=False)
    out_d = nc.declare_dram_parameter("out", [b_loc, n_out, h, w], f32, isOutput=True)

    funnel = []  # final instruction of every proc, for the drain funnel

    with TileContext(nc) as tc:
        with (
            tc.tile_pool(name="const", bufs=1) as cpool,
            tc.tile_pool(name="xp", bufs=1) as xpool,
            tc.tile_pool(name="op", bufs=1) as opool,
            tc.tile_pool(name="ps", bufs=psum_bufs, space="PSUM") as pspool,
            tc.tile_pool(name="jk", bufs=1, space="PSUM") as jkpool,
        ):
            # --- constants (SWDGE lanes; no data deps) -------------------
            wtiles = []
            for g in range(n_groups):
                wt = cpool.tile([128, n_out], xdt, tag=f"w{g}")
                nc.gpsimd.dma_start(out=wt[:], in_=w_d[g * 128:(g + 1) * 128, :])
                wtiles.append(wt)
            btile = cpool.tile([128, o_groups], f32, tag="bias")
            nc.gpsimd.dma_start(out=btile[:], in_=b_d[:])

            # --- PE absorbers for the weight DMA lanes -------------------
            jk = jkpool.tile([32, 512], f32, tag="junk")
            jk_col = [0]

            def absorb(lhsT, rhs, pos):
                nfree = rhs.shape[-1]
                c = jk_col[0]
                jk_col[0] = c + 2
                assert jk_col[0] <= 512
                m = min(lhsT.shape[-1], 32)
                nc.tensor.matmul(jk[0:m, c:c + nfree], lhsT, rhs, start=True,
                                 stop=True, skip_group_check=True,
                                 tile_position=(pos, 0))

            absorb(wtiles[0][0:32, 0:32], wtiles[0][0:32, 32:34], 0)
            absorb(wtiles[0][0:32, 0:32], wtiles[1][0:32, 0:2], 0)

            # bias lane -> ACT: probe on the scalar engine so evictions
            # never wait on the bias DMA.
            ajunk = cpool.tile([128, 4], f32, tag="ajunk")
            nc.scalar.activation(ajunk[0:32, 0:1], btile[0:32, 0:1],
                                 mybir.ActivationFunctionType.Identity)

            # --- main loop ----------------------------------------------
            sw_dmas = []
            last_mm = last_act = None
            xts = []
            for b in range(b_loc):
                xflat = x_d[b]
                xts.append([])
                for g in range(n_groups):
                    xt = xpool.tile([128, epc], xdt, tag=f"x{b}_{g}")
                    xts[b].append(xt)
                    for (off, p0, p1, c0, c1) in segs[g]:
                        nch = p1 - p0
                        span = (KS * (nch - 1) + 1) * epc
                        base = c0 * epc + off
                        src = xflat[base:base + span].rearrange(
                            "(c e) -> c e", e=epc)[::KS, :]
                        if off > 0:
                            # skip the tail spill; it lands in pad columns
                            d = nc.gpsimd.dma_start(
                                out=xt[p0:p1, 0:epc - off], in_=src[:, 0:epc - off])
                        else:
                            d = nc.gpsimd.dma_start(out=xt[p0:p1, :], in_=src)
                        sw_dmas.append(d)
                    # fold this tile's load lanes into PE's observed clock
                    # with tiny absorber matmuls on 32-aligned partition
                    # spans (engine APs must start at partition 0/32/64/96;
                    # absorbers must be on PE itself — the scheduler only
                    # preserves program order within an engine).  Greedy
                    # order: each span adds exactly one not-yet-observed
                    # class DMA, so every absorber needs a single wait.
                    # Column 0 is written by every class DMA.
                    spans = [(s, s + 32) for s in range(0, 128, 32)]
                    seg_of_span = [
                        {i for i, (_, p0, p1, _, _) in enumerate(segs[g])
                         if p0 < s1 and p1 > s0}
                        for (s0, s1) in spans
                    ]
                    observed = set()
                    remaining = list(range(len(spans)))
                    while remaining:
                        pick = next(i for i in remaining
                                    if len(seg_of_span[i] - observed) <= 1)
                        remaining.remove(pick)
                        observed |= seg_of_span[pick]
                        s0, s1 = spans[pick]
                        # N=2: fp32r matmuls need an even innermost count
                        absorb(wtiles[0][s0:s1, 0:32], xt[s0:s1, 0:2], s0)
                    assert observed == set(range(len(segs[g])))

                for og in range(o_groups):
                    ot = opool.tile([128, hw], f32, tag=f"ot{b}_{og}")
                    for n in range(n_chunks):
                        nsl = slice(n * n_free, (n + 1) * n_free)
                        ps = pspool.tile([128, n_free], f32, tag="ps")
                        for g in range(n_groups):
                            lhsT = wtiles[g][:, og * 128:(og + 1) * 128]
                            rhs = xts[b][g][:].rearrange(
                                "p (h w) -> p h w", w=wp)[
                                :, n * rows_per_chunk:(n + 1) * rows_per_chunk,
                                0:w]
                            last_mm = nc.tensor.matmul(
                                ps[:], lhsT, rhs, start=(g == 0),
                                stop=(g == n_groups - 1))
                        # PSUM->SBUF eviction with fused bias on ACT; tile
                        # deps are PE-absorbed, so matmuls only ever wait
                        # on ACT (PSUM recycling), and evictions on PE.
                        last_act = nc.scalar.activation(
                            ot[:, nsl], ps[:],
                            mybir.ActivationFunctionType.Identity,
                            bias=btile[:, og:og + 1])
                    st = nc.sync.dma_start(
                        out=out_d[b, og * 128:(og + 1) * 128].rearrange(
                            "c h w -> c (h w)"),
                        in_=ot[:])
                    funnel.append(st)

            # End-of-kernel drain funnel: SP nops each waiting on one
            # outstanding producer so the drain itself needs no multi-wait.
            funnel.extend(sw_dmas[-8:])
            funnel.append(last_mm)
            funnel.append(last_act)
            for dep in funnel:
                nop = nc.sync.nop(nofuse=True, hint="drain_funnel")
                add_dep_helper(nop.ins, dep.ins, reason="drain funnel")
    return nc


_CACHED_NC = None


def _get_nc():
    global _CACHED_NC
    if _CACHED_NC is None:
        _CACHED_NC = build_nc()
    return _CACHED_NC


def _pad_x(x):
    """[B, C, H, W] -> flat padded [B, C*EPC + 4] (rows padded 56->59)."""
    xp = np.zeros((x.shape[0], C * EPC + 4), np.float32)
    v = xp[:, :C * EPC].reshape(x.shape[0], C, H, WP)
    v[:, :, :, 0:W] = x
    return xp


def run(x, weight, bias, trace=False):
    from concourse.bass_utils import run_bass_kernel_spmd

    perm, _ = _perm_and_segments(C)
    wt = np.ascontiguousarray(weight[:, perm].T)          # [C_perm, O]
    b2 = np.ascontiguousarray(bias.reshape(O // 128, 128).T)  # [128, o_groups]
    xp = _pad_x(np.ascontiguousarray(x, dtype=np.float32))

    nc = _get_nc()
    in_maps = [
        {"x": xp[i * B_LOC:(i + 1) * B_LOC], "wt": wt, "bias": b2}
        for i in range(N_CORES)
    ]
    res = run_bass_kernel_spmd(nc, in_maps, list(range(N_CORES)), trace=trace)
    out = np.concatenate([res.results[i]["out"] for i in range(N_CORES)], axis=0)
    return out, res


def kernel(x, weight, bias):
    out, _ = run(x, weight, bias, trace=False)
    return out
